# revision 41
# baseline (speedup 1.0000x reference)
"""Multi-head attention (B=2, S=2048, D=1024, H=16, A=64) on 8 TRN2 NeuronCores.

Sharding: core c = b*4 + g handles batch b and head-group g (4 heads = 2 pairs).
 - Tensor-parallel over heads: each core computes q/k/v projections for its 4
   heads, attention for those heads, and a partial output projection (its 256
   rows of Wout). The all-reduce over head groups is done host-side during the
   gather (sum of 4 partials per batch).
 - Key/value sequence is mask-compacted host-side: masked keys contribute
   exactly 0 to softmax, so K/V work only runs on valid key positions
   (padded to a multiple of 128; pad slots get an additive -60 bias).

Device-side pipeline (all matmul operands bf16, fp32 PSUM accumulate):
 - Scores computed transposed (scoresT[ki, qi]); mask bias applied for free via
   the ACT engine's per-partition bias on exp.
 - Head PAIRS are processed with PE-array tiling: the two K=64 score matmuls of
   a pair run concurrently in row groups (tile_position (0,0)/(64,0)), and the
   two M=64 PV matmuls run concurrently in col groups ((0,0)/(0,64)) writing
   ctx for head hp=0 to psum partitions 0-63 and hp=1 to 64-127.
 - Softmax denominators come from a separate ones-matmul pass over the probs
   (M=64 broadcast across partitions), then DVE reciprocal + multiply.
 - Emission is software-pipelined so the ACT engine (exp = the critical
   resource at ~83us) never starves: projections / PV / denominators / output
   projection are interleaved into the gaps between score matmuls.
"""

import numpy as np

import concourse.bass as bass
import concourse.bacc as bacc
import concourse.mybir as mybir
import concourse.tile as tile
from concourse.bass_utils import run_bass_kernel_spmd

F32 = mybir.dt.float32
BF16 = mybir.dt.bfloat16
EXP = mybir.ActivationFunctionType.Exp
MULT = mybir.AluOpType.mult

B = 2
S = 2048
D = 1024
H = 16
A = 64
HG = 4           # head groups (cores per batch)
HL = H // HG     # heads per core = 4 (2 pairs)
NP = 2           # pairs per core
DSUB = D // 128  # 8
NEG = -60.0      # additive mask bias; exp(-60) vanishes in fp32 sums
QCH = 512        # matmul moving-dim max


def build_program(C: int, debug: bool = False) -> bass.Bass:
    """Build the per-core SPMD program for key-capacity C (multiple of 128)."""
    assert C % 128 == 0
    KT = C // 128
    QH = 1024 if KT <= 10 else 512   # qi window per scores psum tile
    NH = S // QH                     # number of qi windows ("halves")
    NC2 = QH // QCH                  # 512-chunks per window

    nc = bacc.Bacc("TRN2", target_bir_lowering=False, name=f"mha_c{C}")
    # all inputs are laid out host-side so each SBUF partition's data is one
    # contiguous DRAM run (128 large descriptors per DMA instead of ~1024)
    xq_d = nc.dram_tensor(
        "xq", [128, S // QCH, DSUB, QCH], BF16, kind="ExternalInput"
    )
    xkv_d = nc.dram_tensor(
        "xkv", [128, C // 128, DSUB, 128], BF16, kind="ExternalInput"
    )
    wqkv_d = nc.dram_tensor(
        "wqkv", [128, 3, DSUB, NP * 128], BF16, kind="ExternalInput"
    )
    wout_d = nc.dram_tensor("wout", [128, NP, D], BF16, kind="ExternalInput")
    mb_d = nc.dram_tensor("mbias", [128, C // 128], F32, kind="ExternalInput")
    out_d = nc.dram_tensor("out", [S, D], BF16, kind="ExternalOutput")
    if debug:
        qT_d = nc.dram_tensor("qT_dbg", [128, NP, S], BF16, kind="ExternalOutput")
        kT_d = nc.dram_tensor("kT_dbg", [128, NP, C], BF16, kind="ExternalOutput")
        vx_d = nc.dram_tensor(
            "vx_dbg", [128, KT, NP, 128], BF16, kind="ExternalOutput"
        )
        pr_d = nc.dram_tensor(
            "pr_dbg", [NH, 2, 128, KT, QH], BF16, kind="ExternalOutput"
        )  # pair0 probs for every window
        ctx_d = nc.dram_tensor("ctx_dbg", [128, NP, S], BF16, kind="ExternalOutput")

    # kv chunk plan (in key units): small first chunk for fast pipeline
    # start, then 512s
    kvchunks = []
    off = 0
    first = min(128, C)
    kvchunks.append((off, first))
    off += first
    while off < C:
        w = min(512, C - off)
        kvchunks.append((off, w))
        off += w

    with tile.TileContext(nc) as tc:
        with (
            tc.tile_pool(name="const", bufs=1) as const,
            tc.tile_pool(name="xqp", bufs=4) as xqp,
            tc.tile_pool(name="xkvp", bufs=3) as xkvp,
            tc.tile_pool(name="prp", bufs=5) as prp,
            tc.tile_pool(name="smp", bufs=4) as smp,
            tc.tile_pool(name="rdnp", bufs=3) as rdnp,
            tc.tile_pool(name="otp", bufs=3) as otp,
            tc.tile_pool(name="psS", bufs=2, space="PSUM") as psS,
            tc.tile_pool(name="psPV", bufs=2, space="PSUM") as psPV,
            tc.tile_pool(name="psW", bufs=2, space="PSUM") as psW,
        ):
            # ---------------- persistent SBUF residents ----------------
            w_sb = const.tile([128, 3, DSUB, NP * 128], BF16)
            wout_sb = const.tile([128, NP, D], BF16)
            mb_sb = const.tile([128, KT], F32)
            qT = const.tile([128, NP, S], BF16)      # [hp*64+a, pair, qi]
            kT = const.tile([128, NP, C], BF16)      # [hp*64+a, pair, ki]
            vx = const.tile([128, KT, NP, 128], BF16)  # [ki%128, kt, pair, hp*64+a]
            ctxT = const.tile([128, NP, S], BF16)    # [hp*64+a, pair, qi]
            onesb = const.tile([128, A], BF16)
            nc.vector.memset(onesb, 1.0)


            # ---------------- DMAs (two parallel input queues) ----------------
            # sync queue: weights + xkv + mbias; scalar queue: xq; gpsimd: out.
            # Order tuned so each consumer's data lands just before first use.
            xkv_tiles = [
                xkvp.tile([128, 4, DSUB, 128], BF16, tag="xkv", name=f"xkv{i}")
                for i in range(len(kvchunks))
            ]

            # three parallel input queues; the first q chunk is split across
            # two queues so the first scores window unblocks earlier
            xq_tiles = [
                xqp.tile([128, DSUB, QCH], BF16, tag="xq", name=f"xq{c}")
                for c in range(S // QCH)
            ]

            def dma_xkv(engine, i):
                o0, w = kvchunks[i]
                k0, nk = o0 // 128, w // 128
                engine.dma_start(
                    xkv_tiles[i][:, :nk], xkv_d.ap()[:, k0 : k0 + nk]
                )

            # criticality-ordered across all three queues: the first scores
            # window's data (xkv0, xq0, xq1, w_k, w_q) streams first with the
            # whole wire to itself; everything else queues behind it.
            dma_xkv(nc.gpsimd, 0)
            nc.gpsimd.dma_start(xq_tiles[0][:, 0:4, :], xq_d.ap()[:, 0, 0:4])
            for i in range(1, len(kvchunks)):
                dma_xkv(nc.gpsimd, i)
            nc.scalar.dma_start(xq_tiles[0][:, 4:8, :], xq_d.ap()[:, 0, 4:8])
            for c in range(1, S // QCH):
                nc.scalar.dma_start(xq_tiles[c], xq_d.ap()[:, c])
            nc.sync.dma_start(w_sb[:, 1], wqkv_d.ap()[:, 1])  # k block
            nc.sync.dma_start(mb_sb, mb_d.ap())
            nc.sync.dma_start(w_sb[:, 0], wqkv_d.ap()[:, 0])  # q block
            nc.sync.dma_start(w_sb[:, 2], wqkv_d.ap()[:, 2])  # v block
            nc.sync.dma_start(wout_sb, wout_d.ap())

            # ---------------- emission units ----------------
            def emit_kproj(i):
                o0, w = kvchunks[i]
                nk = w // 128
                xc = xkv_tiles[i]
                for ct in range(NP):
                    ps = psW.tile([128, QCH], F32, tag="psw", name=f"kps{i}_{ct}")
                    for o in range(DSUB):
                        nc.tensor.matmul(
                            ps[:, :w],
                            w_sb[:, 1, o, ct * 128 : (ct + 1) * 128],
                            xc[:, :nk, o, :],
                            start=(o == 0),
                            stop=(o == DSUB - 1),
                        )
                    nc.vector.tensor_copy(out=kT[:, ct, o0 : o0 + w], in_=ps[:, :w])

            def emit_vproj(i):
                o0, w = kvchunks[i]
                xc = xkv_tiles[i]
                for k4 in range(w // 128):
                    kt = (o0 + k4 * 128) // 128
                    ps = psW.tile([128, QCH], F32, tag="psw", name=f"vps{kt}")
                    pv = ps[:, 0 : NP * 128]
                    for o in range(DSUB):
                        nc.tensor.matmul(
                            pv,
                            xc[:, k4, o, :],
                            w_sb[:, 2, o, :],
                            start=(o == 0),
                            stop=(o == DSUB - 1),
                        )
                    nc.vector.tensor_copy(
                        out=vx[:, kt, :, :],
                        in_=pv.rearrange("x (g y) -> x g y", g=2),
                    )

            def emit_qproj(c, p, half=None, box=None):
                xt = xq_tiles[c]
                if half in (None, 0):
                    box[0] = psW.tile(
                        [128, QCH], F32, tag="psw", name=f"qps{c}_{p}"
                    )
                ps = box[0]
                orng = range(DSUB) if half is None else range(
                    half * DSUB // 2, (half + 1) * DSUB // 2
                )
                for o in orng:
                    nc.tensor.matmul(
                        ps,
                        w_sb[:, 0, o, p * 128 : (p + 1) * 128],
                        xt[:, o, :],
                        start=(o == 0),
                        stop=(o == DSUB - 1),
                    )
                if half in (None, 1):
                    nc.vector.tensor_copy(
                        out=qT[:, p, c * QCH : (c + 1) * QCH], in_=ps
                    )

            def emit_scores(p, kt, h):
                """Row-tiled pair of K=64 score matmuls -> two [128, QH] psums."""
                scs = []
                for hp in range(2):
                    sc = psS.tile([128, QH], F32, tag="sc", name=f"sc{h}_{p}_{kt}_{hp}")
                    scs.append(sc)
                rows0 = slice(0, 64)
                rows1 = slice(64, 128)
                for c in range(NC2):
                    q0 = h * QH + c * QCH
                    nc.tensor.matmul(
                        scs[0][:, c * QCH : (c + 1) * QCH],
                        kT[rows0, p, kt * 128 : (kt + 1) * 128],
                        qT[rows0, p, q0 : q0 + QCH],
                        start=True,
                        stop=True,
                    )
                    nc.tensor.matmul(
                        scs[1][:, c * QCH : (c + 1) * QCH],
                        kT[rows1, p, kt * 128 : (kt + 1) * 128],
                        qT[rows1, p, q0 : q0 + QCH],
                        start=True,
                        stop=True,
                    )
                return scs

            def emit_exp(p, kt, scs, prs, psums, split=False):
                """exp on ACT + incremental DVE accumulation of sum_kt(pr)
                (per-head softmax denominator numerator, reduced over kt).
                split=True issues the exp in 512-wide halves so ACT can start
                before the window's second xq chunk has landed."""
                for hp in range(2):
                    if split:
                        for c in range(NC2):
                            nc.scalar.activation(
                                out=prs[hp][:, kt, c * QCH : (c + 1) * QCH],
                                in_=scs[hp][:, c * QCH : (c + 1) * QCH],
                                func=EXP,
                                bias=mb_sb[:, kt : kt + 1],
                                scale=1.0,
                            )
                    else:
                        nc.scalar.activation(
                            out=prs[hp][:, kt, :],
                            in_=scs[hp],
                            func=EXP,
                            bias=mb_sb[:, kt : kt + 1],
                            scale=1.0,
                        )
                    if KT == 1:
                        continue
                    eng = nc.vector
                    if kt == 1:
                        eng.tensor_tensor(
                            psums[hp], prs[hp][:, 0, :], prs[hp][:, 1, :],
                            mybir.AluOpType.add,
                        )
                    elif kt > 1:
                        eng.tensor_tensor(
                            psums[hp], psums[hp], prs[hp][:, kt, :],
                            mybir.AluOpType.add,
                        )

            def emit_pv_kt(pvt, p, kt, c, prs):
                """One kt step of the col-tiled packed PV accumulation."""
                for hp in range(2):
                    nc.tensor.matmul(
                        pvt[hp * 64 : (hp + 1) * 64, :],
                        vx[:, kt, p, hp * 64 : (hp + 1) * 64],
                        prs[hp][:, kt, c * QCH : (c + 1) * QCH],
                        start=(kt == 0),
                        stop=(kt == KT - 1),
                    )

            def emit_den(dent, c, prs, psums):
                """Partition-reduce sum_kt(pr) -> denominator, replicated over
                64 partitions per head (col-tiled pair, one shot each)."""
                for hp in range(2):
                    src = (
                        psums[hp][:, c * QCH : (c + 1) * QCH]
                        if KT > 1
                        else prs[hp][:, 0, c * QCH : (c + 1) * QCH]
                    )
                    nc.tensor.matmul(
                        dent[hp * 64 : (hp + 1) * 64, :],
                        onesb,
                        src,
                        start=True,
                        stop=True,
                    )

            def emit_norm(p, h, c, pvt, dent):
                rden = rdnp.tile([128, QCH], F32, tag="rden")
                nc.vector.reciprocal_approx_fast(rden, dent)
                q0 = h * QH + c * QCH
                nc.vector.tensor_tensor(
                    ctxT[:, p, q0 : q0 + QCH], pvt, rden, MULT
                )

            def emit_outproj(st, drain, dcs=(0, 1)):
                win = st * 128
                for dc in dcs:
                    po = psW.tile([128, QCH], F32, tag="psw", name=f"po{st}_{dc}")
                    for s2 in range(NP):
                        nc.tensor.matmul(
                            po,
                            ctxT[:, s2, win : win + 128],
                            wout_sb[:, s2, dc * QCH : (dc + 1) * QCH],
                            start=(s2 == 0),
                            stop=(s2 == NP - 1),
                        )
                    ot = otp.tile([128, QCH], BF16, tag="ot")
                    if drain == "act":
                        nc.scalar.copy(out=ot, in_=po)
                    else:
                        nc.vector.tensor_copy(out=ot, in_=po)
                    nc.gpsimd.dma_start(
                        out_d.ap()[win : win + 128, dc * QCH : (dc + 1) * QCH], ot
                    )

            # -------- background queue: (cost_us, closure) FIFO --------
            # credit pacing: each kt step grants STEP_CREDIT us of PE slack;
            # units pop only while credit is positive (carries debt across
            # steps so an oversized unit skips the next pops).
            queue = []
            qout = []   # outproj-only units: safe to pop while psPV is held
            credit = [0.0]
            STEP_CREDIT = 1.35

            def pop_step(boost=1.0, out_only=False):
                credit[0] += STEP_CREDIT * boost
                while (queue or qout) and credit[0] > 0:
                    if queue and not out_only:
                        cost, fn = queue.pop(0)
                    elif qout:
                        cost, fn = qout.pop(0)
                    else:
                        break
                    fn()
                    credit[0] -= cost
                if not queue and not qout:
                    credit[0] = 0.0

            def pop_all(out_only=False):
                while queue and not out_only:
                    _, fn = queue.pop(0)
                    fn()
                while qout:
                    _, fn = qout.pop(0)
                    fn()
                credit[0] = 0.0

            KPROJ_COST = lambda w: 16 * max(w / 2.4, 110.0) / 1000.0
            VPROJ_COST = lambda w: (w // 128) * 8 * 110.0 / 1000.0
            QPROJ_COST = 8 * 250.0 / 1000.0
            PV_COST = KT * 216.0 / 1000.0
            OUTPJ_COST = 2 * NP * 250.0 / 1000.0

            # ---------------- startup ----------------
            emit_kproj(0)
            for c in range(NC2):
                emit_qproj(c, 0, box=[None])

            for i in range(1, len(kvchunks)):
                queue.append((KPROJ_COST(kvchunks[i][1]), lambda i=i: emit_kproj(i)))
            queue.append((VPROJ_COST(kvchunks[0][1]), lambda: emit_vproj(0)))
            def queue_qproj(c, p):
                box = [None]
                for half in range(2):
                    queue.append(
                        (
                            QPROJ_COST / 2,
                            lambda c=c, p=p, half=half, box=box: emit_qproj(
                                c, p, half, box
                            ),
                        )
                    )

            for c in range(NC2):
                queue_qproj(c, 1)
            for i in range(1, len(kvchunks)):
                queue.append((VPROJ_COST(kvchunks[i][1]), lambda i=i: emit_vproj(i)))
            for c in range(NC2, S // QCH):
                for p in range(NP):
                    queue_qproj(c, p)

            # ---------------- main loop ----------------
            for h in range(NH):
                for p in range(NP):
                    is_final = h == NH - 1 and p == NP - 1
                    prs = [
                        prp.tile([128, KT, QH], BF16, tag="pr", name=f"pr{h}_{p}_{hp}")
                        for hp in range(2)
                    ]
                    psums = [
                        smp.tile([128, QH], BF16, tag="psum", name=f"psm{h}_{p}_{hp}")
                        for hp in range(2)
                    ]
                    if is_final:
                        # kt-major PV to minimize the post-exp tail. The pv
                        # psum tiles are held across the loop, so the queue
                        # must fully drain BEFORE they are allocated (popped
                        # units allocate from psPV/psW and would deadlock
                        # behind held tiles) -> lazy alloc once queue empties.
                        pvts = None
                        done_kt = 0
                        for kt in range(KT):
                            scs = emit_scores(p, kt, h)
                            emit_exp(p, kt, scs, prs, psums)
                            if pvts is not None:
                                pop_step(out_only=True)
                            else:
                                pop_step(1.5)
                                if not queue:
                                    pvts = [
                                        psPV.tile(
                                            [128, QCH], F32, tag="pv",
                                            name=f"pvF{c}",
                                        )
                                        for c in range(NC2)
                                    ]
                            if pvts is not None:
                                stop_kt = min(kt, done_kt + 3)
                                while done_kt < stop_kt:
                                    for c in range(NC2):
                                        emit_pv_kt(pvts[c], p, done_kt, c, prs)
                                    done_kt += 1
                        pop_all(out_only=pvts is not None)
                        if pvts is None:
                            pvts = [
                                psPV.tile([128, QCH], F32, tag="pv", name=f"pvF{c}")
                                for c in range(NC2)
                            ]
                        while done_kt < KT:
                            for c in range(NC2):
                                emit_pv_kt(pvts[c], p, done_kt, c, prs)
                            done_kt += 1
                        for c in range(NC2):
                            dent = psW.tile(
                                [128, QCH], F32, tag="psw", name=f"denF{c}"
                            )
                            emit_den(dent, c, prs, psums)
                            emit_norm(p, h, c, pvts[c], dent)
                            for st in range(4):
                                stg = (h * QH + c * QCH) // 128 + st
                                emit_outproj(stg, "act" if st % 2 else "vec")
                    else:
                        boost = 1.5 if h == NH - 1 else (1.2 if p == 1 else 1.0)
                        for kt in range(KT):
                            scs = emit_scores(p, kt, h)
                            emit_exp(
                                p, kt, scs, prs, psums,
                                split=(h == 0 and p == 0 and kt < 2),
                            )
                            pop_step(boost)
                        if debug and p == 0:
                            for hp in range(2):
                                nc.sync.dma_start(pr_d.ap()[h, hp], prs[hp])
                        # gap work -> absorbed into the next pair's kt steps
                        for c in range(NC2):
                            box = [None]

                            def run_pv(box=box, p=p, c=c, prs=prs):
                                pvt = psPV.tile(
                                    [128, QCH], F32, tag="pv", name=f"pv{p}_{c}"
                                )
                                for kt in range(KT):
                                    emit_pv_kt(pvt, p, kt, c, prs)
                                box[0] = pvt

                            def run_den(box=box, p=p, h=h, c=c, prs=prs,
                                        psums=psums):
                                dent = psW.tile(
                                    [128, QCH], F32, tag="psw", name=f"den{p}_{c}"
                                )
                                emit_den(dent, c, prs, psums)
                                emit_norm(p, h, c, box[0], dent)

                            queue.append((PV_COST, run_pv))
                            queue.append((0.6, run_den))
                        if p == NP - 1:
                            # output projection for this window (needs both pairs)
                            for st in range(QH // 128):
                                stg = h * (QH // 128) + st
                                for dc in range(2):
                                    qout.append(
                                        (
                                            OUTPJ_COST / 2,
                                            lambda stg=stg, dc=dc: emit_outproj(
                                                stg, "vec", (dc,)
                                            ),
                                        )
                                    )
            pop_all()  # flush
            if debug:
                nc.sync.dma_start(qT_d.ap(), qT)
                nc.sync.dma_start(kT_d.ap(), kT)
                nc.sync.dma_start(vx_d.ap(), vx)
                nc.sync.dma_start(ctx_d.ap(), ctxT)

    return nc


_PROGRAM_CACHE: dict[int, bass.Bass] = {}


def _get_program(C: int) -> bass.Bass:
    if C not in _PROGRAM_CACHE:
        nc = build_program(C)
        nc.finalize()
        _PROGRAM_CACHE[C] = nc
    return _PROGRAM_CACHE[C]


def _ceil128(n: int) -> int:
    return max(128, (n + 127) // 128 * 128)


def prepare_in_maps(qs, mask, Wqkv, Wout):
    """Shard FULL inputs into 8 per-core input maps. Returns (in_maps, C)."""
    import ml_dtypes

    np_mm = ml_dtypes.bfloat16
    qs = np.ascontiguousarray(qs, dtype=np.float32)
    mask = np.asarray(mask)
    Wqkv = np.ascontiguousarray(Wqkv, dtype=np.float32)
    Wout = np.ascontiguousarray(Wout, dtype=np.float32)

    nvalid = [int(np.count_nonzero(mask[b])) for b in range(B)]
    if min(nvalid) == 0:
        C = S  # degenerate masks: run dense
    else:
        C = min(S, _ceil128(max(nvalid)))
    compact = C < S

    # device layouts put each SBUF partition's data contiguous in DRAM:
    #   xq[p, c, o, j]  = x[o*128+p, c*512+j]
    #   xkv[p, k, o, j] = xkv_compact[o*128+p, k*128+j]
    #   mb[p, t]        = bias[t*128 + p]
    def to_dev(xT, inner):
        return np.ascontiguousarray(
            xT.reshape(DSUB, 128, xT.shape[1] // inner, inner)
            .transpose(1, 2, 0, 3)
            .astype(np_mm)
        )

    xq, xkv, mb = [], [], []
    for b in range(B):
        xq.append(to_dev(qs[b].T, QCH))
        if compact:
            idx = np.nonzero(mask[b] != 0)[0]
            sel = np.concatenate(
                [idx, np.zeros(C - len(idx), dtype=idx.dtype)]
            )
            bias = np.full(C, NEG, dtype=np.float32)
            bias[: len(idx)] = 0.0
            xkv.append(to_dev(qs[b][sel].T, 128))
        else:
            bias = np.where(mask[b] != 0, 0.0, NEG).astype(np.float32)
            xkv.append(to_dev(qs[b].T, 128))
        mb.append(np.ascontiguousarray(bias.reshape(C // 128, 128).T))

    in_maps = []
    for b in range(B):
        for g in range(HG):
            h0 = g * HL
            wq = Wqkv[:, (0 * H + h0) * A : (0 * H + h0 + HL) * A] * (
                1.0 / np.sqrt(np.float32(A))
            )
            wk = Wqkv[:, (1 * H + h0) * A : (1 * H + h0 + HL) * A]
            wv = Wqkv[:, (2 * H + h0) * A : (2 * H + h0 + HL) * A]
            # wqkv[p, blk, o, j] = blk_weights[o*128+p, j]
            wqkv_s = np.ascontiguousarray(
                np.stack([wq, wk, wv], axis=1)       # [D, 3, 256]
                .reshape(DSUB, 128, 3, HL * A)
                .transpose(1, 2, 0, 3)
                .astype(np_mm)
            )
            # wout[p, pair, d] = Wout_slice[pair*128 + p, d]
            wout_s = np.ascontiguousarray(
                Wout[h0 * A : (h0 + HL) * A, :]
                .reshape(NP, 128, D)
                .transpose(1, 0, 2)
                .astype(np_mm)
            )
            in_maps.append(
                {
                    "xq": xq[b],
                    "xkv": xkv[b],
                    "wqkv": wqkv_s,
                    "wout": wout_s,
                    "mbias": mb[b],
                }
            )
    return in_maps, C


def gather_output(results, bout):
    """Sum the 4 head-group partials per batch and add bout."""
    out = np.empty((B, S, D), dtype=np.float32)
    for b in range(B):
        acc = results[b * HG]["out"].astype(np.float32).copy()
        for g in range(1, HG):
            acc += results[b * HG + g]["out"]
        out[b] = acc + bout.astype(np.float32)[None, :]
    return out


def _ensure_ntff_hook():
    """Inject antenv.axon_hooks (missing on this image) so trace=True works."""
    import sys
    import types

    try:
        from antenv import axon_hooks  # noqa: F401
        return
    except ImportError:
        pass
    mod = types.ModuleType("antenv.axon_hooks")
    _h = [None]
    mod.set_axon_ntff_profile_hook = lambda h: _h.__setitem__(0, h)
    mod.get_axon_ntff_profile_hook = lambda: _h[0]
    sys.modules["antenv.axon_hooks"] = mod
    import antenv

    antenv.axon_hooks = mod
    try:
        from trn_agent_boot.trn_boot import _ntff_profile_via_ctypes

        mod.set_axon_ntff_profile_hook(
            _ntff_profile_via_ctypes("/opt/axon/libaxon_pjrt.so")
        )
    except Exception:
        pass


def run(qs, mask, Wqkv, Wout, bout, trace=False):
    if trace:
        _ensure_ntff_hook()
    in_maps, C = prepare_in_maps(qs, mask, Wqkv, Wout)
    nc = _get_program(C)
    res = run_bass_kernel_spmd(
        nc, in_maps, core_ids=list(range(B * HG)), trace=trace
    )
    return gather_output(res.results, np.asarray(bout)), res


def kernel(qs, mask, Wqkv, Wout, bout):
    return run(qs, mask, Wqkv, Wout, bout, trace=False)[0]


# revision 42
# speedup vs baseline: 1.1517x; 1.1517x over previous
"""Multi-head attention (B=2, S=2048, D=1024, H=16, A=64) on 8 TRN2 NeuronCores.

Sharding: core c = b*4 + g handles batch b and head-group g (4 heads = 2 pairs).
 - Tensor-parallel over heads: each core computes q/k/v projections for its 4
   heads, attention for those heads, and a partial output projection (its 256
   rows of Wout). The all-reduce over head groups is done host-side during the
   gather (sum of 4 partials per batch).
 - Key/value sequence is mask-compacted host-side: masked keys contribute
   exactly 0 to softmax, so K/V work only runs on valid key positions
   (padded to a multiple of 128; pad slots get an additive -60 bias).

Device-side pipeline (all matmul operands bf16, fp32 PSUM accumulate):
 - Scores computed transposed (scoresT[ki, qi]); mask bias applied for free via
   the ACT engine's per-partition bias on exp.
 - Head PAIRS are processed with PE-array tiling: the two K=64 score matmuls of
   a pair run concurrently in row groups (tile_position (0,0)/(64,0)), and the
   two M=64 PV matmuls run concurrently in col groups ((0,0)/(0,64)) writing
   ctx for head hp=0 to psum partitions 0-63 and hp=1 to 64-127.
 - Softmax denominators come from a separate ones-matmul pass over the probs
   (M=64 broadcast across partitions), then DVE reciprocal + multiply.
 - Emission is software-pipelined so the ACT engine (exp = the critical
   resource at ~83us) never starves: projections / PV / denominators / output
   projection are interleaved into the gaps between score matmuls.
"""

import numpy as np

import concourse.bass as bass
import concourse.bacc as bacc
import concourse.mybir as mybir
import concourse.tile as tile
from concourse.bass_utils import run_bass_kernel_spmd

F32 = mybir.dt.float32
BF16 = mybir.dt.bfloat16
EXP = mybir.ActivationFunctionType.Exp
MULT = mybir.AluOpType.mult

B = 2
S = 2048
D = 1024
H = 16
A = 64
HG = 4           # head groups (cores per batch)
HL = H // HG     # heads per core = 4 (2 pairs)
NP = 2           # pairs per core
DSUB = D // 128  # 8
NEG = -60.0      # additive mask bias; exp(-60) vanishes in fp32 sums
QCH = 512        # matmul moving-dim max


def build_program(C: int, debug: bool = False) -> bass.Bass:
    """Build the per-core SPMD program for key-capacity C (multiple of 128)."""
    assert C % 128 == 0
    KT = C // 128
    QH = 1024 if KT <= 10 else 512   # qi window per scores psum tile
    NH = S // QH                     # number of qi windows ("halves")
    NC2 = QH // QCH                  # 512-chunks per window

    nc = bacc.Bacc("TRN2", target_bir_lowering=False, name=f"mha_c{C}")
    # all inputs are laid out host-side so each SBUF partition's data is one
    # contiguous DRAM run (128 large descriptors per DMA instead of ~1024)
    xq_d = nc.dram_tensor(
        "xq", [128, S // QCH, DSUB, QCH], BF16, kind="ExternalInput"
    )
    xkv_d = nc.dram_tensor(
        "xkv", [128, C // 128, DSUB, 128], BF16, kind="ExternalInput"
    )
    wqkv_d = nc.dram_tensor(
        "wqkv", [128, 3, DSUB, NP * 128], BF16, kind="ExternalInput"
    )
    wout_d = nc.dram_tensor("wout", [128, NP, D], BF16, kind="ExternalInput")
    mb_d = nc.dram_tensor("mbias", [128, C // 128], F32, kind="ExternalInput")
    out_d = nc.dram_tensor("out", [S, D], BF16, kind="ExternalOutput")
    if debug:
        qT_d = nc.dram_tensor("qT_dbg", [128, NP, S], BF16, kind="ExternalOutput")
        kT_d = nc.dram_tensor("kT_dbg", [128, NP, C], BF16, kind="ExternalOutput")
        vx_d = nc.dram_tensor(
            "vx_dbg", [128, KT, NP, 128], BF16, kind="ExternalOutput"
        )
        pr_d = nc.dram_tensor(
            "pr_dbg", [NH, 2, 128, KT, QH], BF16, kind="ExternalOutput"
        )  # pair0 probs for every window
        ctx_d = nc.dram_tensor("ctx_dbg", [128, NP, S], BF16, kind="ExternalOutput")

    # kv chunk plan (in key units): small first chunk for fast pipeline
    # start, then 512s
    kvchunks = []
    off = 0
    first = min(128, C)
    kvchunks.append((off, first))
    off += first
    while off < C:
        w = min(512, C - off)
        kvchunks.append((off, w))
        off += w

    with tile.TileContext(nc) as tc:
        with (
            tc.tile_pool(name="const", bufs=1) as const,
            tc.tile_pool(name="xqp", bufs=4) as xqp,
            tc.tile_pool(name="xkvp", bufs=3) as xkvp,
            tc.tile_pool(name="prp", bufs=5) as prp,
            tc.tile_pool(name="smp", bufs=4) as smp,
            tc.tile_pool(name="rdnp", bufs=3) as rdnp,
            tc.tile_pool(name="otp", bufs=3) as otp,
            tc.tile_pool(name="psS", bufs=2, space="PSUM") as psS,
            tc.tile_pool(name="psPV", bufs=2, space="PSUM") as psPV,
            tc.tile_pool(name="psW", bufs=2, space="PSUM") as psW,
        ):
            # ---------------- persistent SBUF residents ----------------
            w_sb = const.tile([128, 3, DSUB, NP * 128], BF16)
            wout_sb = const.tile([128, NP, D], BF16)
            mb_sb = const.tile([128, KT], F32)
            qT = const.tile([128, NP, S], BF16)      # [hp*64+a, pair, qi]
            kT = const.tile([128, NP, C], BF16)      # [hp*64+a, pair, ki]
            vx = const.tile([128, KT, NP, 128], BF16)  # [ki%128, kt, pair, hp*64+a]
            ctxT = const.tile([128, NP, S], BF16)    # [hp*64+a, pair, qi]
            onesb = const.tile([128, A], BF16)
            nc.vector.memset(onesb, 1.0)


            # ---------------- DMAs (two parallel input queues) ----------------
            # sync queue: weights + xkv + mbias; scalar queue: xq; gpsimd: out.
            # Order tuned so each consumer's data lands just before first use.
            xkv_tiles = [
                xkvp.tile([128, 4, DSUB, 128], BF16, tag="xkv", name=f"xkv{i}")
                for i in range(len(kvchunks))
            ]

            # three parallel input queues; the first q chunk is split across
            # two queues so the first scores window unblocks earlier
            xq_tiles = [
                xqp.tile([128, DSUB, QCH], BF16, tag="xq", name=f"xq{c}")
                for c in range(S // QCH)
            ]

            def dma_xkv(engine, i):
                o0, w = kvchunks[i]
                k0, nk = o0 // 128, w // 128
                engine.dma_start(
                    xkv_tiles[i][:, :nk], xkv_d.ap()[:, k0 : k0 + nk]
                )

            # criticality-ordered across all three queues: the first scores
            # window's data (xkv0, xq0, xq1, w_k, w_q) streams first with the
            # whole wire to itself; everything else queues behind it.
            dma_xkv(nc.gpsimd, 0)
            nc.gpsimd.dma_start(xq_tiles[0][:, 0:4, :], xq_d.ap()[:, 0, 0:4])
            for i in range(1, len(kvchunks)):
                dma_xkv(nc.gpsimd, i)
            nc.scalar.dma_start(xq_tiles[0][:, 4:8, :], xq_d.ap()[:, 0, 4:8])
            for c in range(1, S // QCH):
                nc.scalar.dma_start(xq_tiles[c], xq_d.ap()[:, c])
            nc.sync.dma_start(w_sb[:, 1], wqkv_d.ap()[:, 1])  # k block
            nc.sync.dma_start(mb_sb, mb_d.ap())
            nc.sync.dma_start(w_sb[:, 0], wqkv_d.ap()[:, 0])  # q block
            nc.sync.dma_start(w_sb[:, 2], wqkv_d.ap()[:, 2])  # v block
            nc.sync.dma_start(wout_sb, wout_d.ap())

            # ---------------- emission units ----------------
            def emit_kproj(i):
                o0, w = kvchunks[i]
                nk = w // 128
                xc = xkv_tiles[i]
                for ct in range(NP):
                    ps = psW.tile([128, QCH], F32, tag="psw", name=f"kps{i}_{ct}")
                    for o in range(DSUB):
                        nc.tensor.matmul(
                            ps[:, :w],
                            w_sb[:, 1, o, ct * 128 : (ct + 1) * 128],
                            xc[:, :nk, o, :],
                            start=(o == 0),
                            stop=(o == DSUB - 1),
                        )
                    nc.vector.tensor_copy(out=kT[:, ct, o0 : o0 + w], in_=ps[:, :w])

            def emit_vproj(i):
                o0, w = kvchunks[i]
                xc = xkv_tiles[i]
                for k4 in range(w // 128):
                    kt = (o0 + k4 * 128) // 128
                    ps = psW.tile([128, QCH], F32, tag="psw", name=f"vps{kt}")
                    pv = ps[:, 0 : NP * 128]
                    for o in range(DSUB):
                        nc.tensor.matmul(
                            pv,
                            xc[:, k4, o, :],
                            w_sb[:, 2, o, :],
                            start=(o == 0),
                            stop=(o == DSUB - 1),
                        )
                    nc.vector.tensor_copy(
                        out=vx[:, kt, :, :],
                        in_=pv.rearrange("x (g y) -> x g y", g=2),
                    )

            def emit_qproj(c, p, half=None, box=None):
                xt = xq_tiles[c]
                if half in (None, 0):
                    box[0] = psW.tile(
                        [128, QCH], F32, tag="psw", name=f"qps{c}_{p}"
                    )
                ps = box[0]
                orng = range(DSUB) if half is None else range(
                    half * DSUB // 2, (half + 1) * DSUB // 2
                )
                for o in orng:
                    nc.tensor.matmul(
                        ps,
                        w_sb[:, 0, o, p * 128 : (p + 1) * 128],
                        xt[:, o, :],
                        start=(o == 0),
                        stop=(o == DSUB - 1),
                    )
                if half in (None, 1):
                    nc.vector.tensor_copy(
                        out=qT[:, p, c * QCH : (c + 1) * QCH], in_=ps
                    )

            def emit_scores(p, kt, h):
                """Row-tiled pair of K=64 score matmuls -> two [128, QH] psums."""
                scs = []
                for hp in range(2):
                    sc = psS.tile([128, QH], F32, tag="sc", name=f"sc{h}_{p}_{kt}_{hp}")
                    scs.append(sc)
                rows0 = slice(0, 64)
                rows1 = slice(64, 128)
                for c in range(NC2):
                    q0 = h * QH + c * QCH
                    nc.tensor.matmul(
                        scs[0][:, c * QCH : (c + 1) * QCH],
                        kT[rows0, p, kt * 128 : (kt + 1) * 128],
                        qT[rows0, p, q0 : q0 + QCH],
                        start=True,
                        stop=True,
                    )
                    nc.tensor.matmul(
                        scs[1][:, c * QCH : (c + 1) * QCH],
                        kT[rows1, p, kt * 128 : (kt + 1) * 128],
                        qT[rows1, p, q0 : q0 + QCH],
                        start=True,
                        stop=True,
                    )
                return scs

            def emit_exp(p, kt, scs, prs, psums, split=False):
                """exp on ACT + incremental DVE accumulation of sum_kt(pr)
                (per-head softmax denominator numerator, reduced over kt).
                split=True issues the exp in 512-wide halves so ACT can start
                before the window's second xq chunk has landed."""
                for hp in range(2):
                    if split:
                        for c in range(NC2):
                            nc.scalar.activation(
                                out=prs[hp][:, kt, c * QCH : (c + 1) * QCH],
                                in_=scs[hp][:, c * QCH : (c + 1) * QCH],
                                func=EXP,
                                bias=mb_sb[:, kt : kt + 1],
                                scale=1.0,
                            )
                    else:
                        nc.scalar.activation(
                            out=prs[hp][:, kt, :],
                            in_=scs[hp],
                            func=EXP,
                            bias=mb_sb[:, kt : kt + 1],
                            scale=1.0,
                        )
                    if KT == 1:
                        continue
                    eng = nc.vector
                    if kt == 1:
                        eng.tensor_tensor(
                            psums[hp], prs[hp][:, 0, :], prs[hp][:, 1, :],
                            mybir.AluOpType.add,
                        )
                    elif kt > 1:
                        eng.tensor_tensor(
                            psums[hp], psums[hp], prs[hp][:, kt, :],
                            mybir.AluOpType.add,
                        )

            def emit_pv_kt(pvt, p, kt, c, prs):
                """One kt step of the col-tiled packed PV accumulation."""
                for hp in range(2):
                    nc.tensor.matmul(
                        pvt[hp * 64 : (hp + 1) * 64, :],
                        vx[:, kt, p, hp * 64 : (hp + 1) * 64],
                        prs[hp][:, kt, c * QCH : (c + 1) * QCH],
                        start=(kt == 0),
                        stop=(kt == KT - 1),
                    )

            def emit_den(dent, c, prs, psums):
                """Partition-reduce sum_kt(pr) -> denominator, replicated over
                64 partitions per head (col-tiled pair, one shot each)."""
                for hp in range(2):
                    src = (
                        psums[hp][:, c * QCH : (c + 1) * QCH]
                        if KT > 1
                        else prs[hp][:, 0, c * QCH : (c + 1) * QCH]
                    )
                    nc.tensor.matmul(
                        dent[hp * 64 : (hp + 1) * 64, :],
                        onesb,
                        src,
                        start=True,
                        stop=True,
                    )

            def emit_norm(p, h, c, pvt, dent):
                rden = rdnp.tile([128, QCH], F32, tag="rden")
                nc.vector.reciprocal_approx_fast(rden, dent)
                q0 = h * QH + c * QCH
                nc.vector.tensor_tensor(
                    ctxT[:, p, q0 : q0 + QCH], pvt, rden, MULT
                )

            def emit_outproj(st, drain, dcs=(0, 1)):
                win = st * 128
                for dc in dcs:
                    po = psW.tile([128, QCH], F32, tag="psw", name=f"po{st}_{dc}")
                    for s2 in range(NP):
                        nc.tensor.matmul(
                            po,
                            ctxT[:, s2, win : win + 128],
                            wout_sb[:, s2, dc * QCH : (dc + 1) * QCH],
                            start=(s2 == 0),
                            stop=(s2 == NP - 1),
                        )
                    ot = otp.tile([128, QCH], BF16, tag="ot")
                    if drain == "act":
                        nc.scalar.copy(out=ot, in_=po)
                    else:
                        nc.vector.tensor_copy(out=ot, in_=po)
                    nc.gpsimd.dma_start(
                        out_d.ap()[win : win + 128, dc * QCH : (dc + 1) * QCH], ot
                    )

            # -------- background queue: (cost_us, closure) FIFO --------
            # credit pacing: each kt step grants STEP_CREDIT us of PE slack;
            # units pop only while credit is positive (carries debt across
            # steps so an oversized unit skips the next pops).
            queue = []
            qout = []   # outproj-only units: safe to pop while psPV is held
            credit = [0.0]
            STEP_CREDIT = 1.35

            def pop_step(boost=1.0, out_only=False):
                credit[0] += STEP_CREDIT * boost
                while (queue or qout) and credit[0] > 0:
                    if queue and not out_only:
                        cost, fn = queue.pop(0)[:2]
                    elif qout:
                        cost, fn = qout.pop(0)
                    else:
                        break
                    fn()
                    credit[0] -= cost
                if not queue and not qout:
                    credit[0] = 0.0

            def pop_all(out_only=False):
                while queue and not out_only:
                    fn = queue.pop(0)[1]
                    fn()
                while qout:
                    _, fn = qout.pop(0)
                    fn()
                credit[0] = 0.0

            def drain_window_deps(h):
                """Force-pop the FIFO until no qproj unit for window <= h
                remains: the next window's scores MUST be emitted after the
                qT writes they read (emission-order reads return garbage)."""
                def pending():
                    return any(
                        len(e) > 2 and e[2] is not None and e[2] <= h
                        for e in queue
                    )
                while pending():
                    cost, fn = queue.pop(0)[:2]
                    fn()
                    credit[0] -= cost

            KPROJ_COST = lambda w: 16 * max(w / 2.4, 110.0) / 1000.0
            VPROJ_COST = lambda w: (w // 128) * 8 * 110.0 / 1000.0
            QPROJ_COST = 8 * 250.0 / 1000.0
            PV_COST = KT * 216.0 / 1000.0
            OUTPJ_COST = 2 * NP * 250.0 / 1000.0

            # ---------------- startup ----------------
            emit_kproj(0)
            for c in range(NC2):
                emit_qproj(c, 0, box=[None])

            for i in range(1, len(kvchunks)):
                queue.append((KPROJ_COST(kvchunks[i][1]), lambda i=i: emit_kproj(i)))
            queue.append((VPROJ_COST(kvchunks[0][1]), lambda: emit_vproj(0)))
            def queue_qproj(c, p):
                box = [None]
                win = (c * QCH) // QH
                for half in range(2):
                    queue.append(
                        (
                            QPROJ_COST / 2,
                            lambda c=c, p=p, half=half, box=box: emit_qproj(
                                c, p, half, box
                            ),
                            win,
                        )
                    )

            for c in range(NC2):
                queue_qproj(c, 1)
            for i in range(1, len(kvchunks)):
                queue.append((VPROJ_COST(kvchunks[i][1]), lambda i=i: emit_vproj(i)))

            # ---------------- main loop ----------------
            for h in range(NH):
                if h > 0:
                    drain_window_deps(h)
                for p in range(NP):
                    is_final = h == NH - 1 and p == NP - 1
                    prs = [
                        prp.tile([128, KT, QH], BF16, tag="pr", name=f"pr{h}_{p}_{hp}")
                        for hp in range(2)
                    ]
                    psums = [
                        smp.tile([128, QH], BF16, tag="psum", name=f"psm{h}_{p}_{hp}")
                        for hp in range(2)
                    ]
                    if is_final:
                        # kt-major PV to minimize the post-exp tail. The pv
                        # psum tiles are held across the loop, so the queue
                        # must fully drain BEFORE they are allocated (popped
                        # units allocate from psPV/psW and would deadlock
                        # behind held tiles) -> lazy alloc once queue empties.
                        pvts = None
                        done_kt = 0
                        for kt in range(KT):
                            scs = emit_scores(p, kt, h)
                            emit_exp(p, kt, scs, prs, psums)
                            if pvts is not None:
                                pop_step(out_only=True)
                            else:
                                pop_step(1.5)
                                if not queue:
                                    pvts = [
                                        psPV.tile(
                                            [128, QCH], F32, tag="pv",
                                            name=f"pvF{c}",
                                        )
                                        for c in range(NC2)
                                    ]
                            if pvts is not None:
                                stop_kt = min(kt, done_kt + 3)
                                while done_kt < stop_kt:
                                    for c in range(NC2):
                                        emit_pv_kt(pvts[c], p, done_kt, c, prs)
                                    done_kt += 1
                        pop_all(out_only=pvts is not None)
                        if pvts is None:
                            pvts = [
                                psPV.tile([128, QCH], F32, tag="pv", name=f"pvF{c}")
                                for c in range(NC2)
                            ]
                        while done_kt < KT:
                            for c in range(NC2):
                                emit_pv_kt(pvts[c], p, done_kt, c, prs)
                            done_kt += 1
                        for c in range(NC2):
                            dent = psW.tile(
                                [128, QCH], F32, tag="psw", name=f"denF{c}"
                            )
                            emit_den(dent, c, prs, psums)
                            emit_norm(p, h, c, pvts[c], dent)
                            for st in range(4):
                                stg = (h * QH + c * QCH) // 128 + st
                                emit_outproj(stg, "act" if st % 2 else "vec")
                    else:
                        boost = 1.5 if h == NH - 1 else (1.2 if p == 1 else 1.0)
                        for kt in range(KT):
                            scs = emit_scores(p, kt, h)
                            emit_exp(
                                p, kt, scs, prs, psums,
                                split=(h == 0 and p == 0 and kt < 2),
                            )
                            pop_step(boost)
                        if debug and p == 0:
                            for hp in range(2):
                                nc.sync.dma_start(pr_d.ap()[h, hp], prs[hp])
                        # gap work -> absorbed into the next pair's kt steps
                        for c in range(NC2):
                            box = [None]

                            def run_pv(box=box, p=p, c=c, prs=prs):
                                pvt = psPV.tile(
                                    [128, QCH], F32, tag="pv", name=f"pv{p}_{c}"
                                )
                                for kt in range(KT):
                                    emit_pv_kt(pvt, p, kt, c, prs)
                                box[0] = pvt

                            def run_den(box=box, p=p, h=h, c=c, prs=prs,
                                        psums=psums):
                                dent = psW.tile(
                                    [128, QCH], F32, tag="psw", name=f"den{p}_{c}"
                                )
                                emit_den(dent, c, prs, psums)
                                emit_norm(p, h, c, box[0], dent)

                            queue.append((PV_COST, run_pv))
                            queue.append((0.6, run_den))
                        if h == 0 and p == 0:
                            for cq in range(NC2, S // QCH):
                                for pq in range(NP):
                                    queue_qproj(cq, pq)
                        if p == NP - 1:
                            # output projection for this window (needs both pairs)
                            for st in range(QH // 128):
                                stg = h * (QH // 128) + st
                                for dc in range(2):
                                    qout.append(
                                        (
                                            OUTPJ_COST / 2,
                                            lambda stg=stg, dc=dc: emit_outproj(
                                                stg, "vec", (dc,)
                                            ),
                                        )
                                    )
            pop_all()  # flush
            if debug:
                nc.sync.dma_start(qT_d.ap(), qT)
                nc.sync.dma_start(kT_d.ap(), kT)
                nc.sync.dma_start(vx_d.ap(), vx)
                nc.sync.dma_start(ctx_d.ap(), ctxT)

    return nc


_PROGRAM_CACHE: dict[int, bass.Bass] = {}


def _get_program(C: int) -> bass.Bass:
    if C not in _PROGRAM_CACHE:
        nc = build_program(C)
        nc.finalize()
        _PROGRAM_CACHE[C] = nc
    return _PROGRAM_CACHE[C]


def _ceil128(n: int) -> int:
    return max(128, (n + 127) // 128 * 128)


def prepare_in_maps(qs, mask, Wqkv, Wout):
    """Shard FULL inputs into 8 per-core input maps. Returns (in_maps, C)."""
    import ml_dtypes

    np_mm = ml_dtypes.bfloat16
    qs = np.ascontiguousarray(qs, dtype=np.float32)
    mask = np.asarray(mask)
    Wqkv = np.ascontiguousarray(Wqkv, dtype=np.float32)
    Wout = np.ascontiguousarray(Wout, dtype=np.float32)

    nvalid = [int(np.count_nonzero(mask[b])) for b in range(B)]
    if min(nvalid) == 0:
        C = S  # degenerate masks: run dense
    else:
        C = min(S, _ceil128(max(nvalid)))
    compact = C < S

    # device layouts put each SBUF partition's data contiguous in DRAM:
    #   xq[p, c, o, j]  = x[o*128+p, c*512+j]
    #   xkv[p, k, o, j] = xkv_compact[o*128+p, k*128+j]
    #   mb[p, t]        = bias[t*128 + p]
    def to_dev(xT, inner):
        return np.ascontiguousarray(
            xT.reshape(DSUB, 128, xT.shape[1] // inner, inner)
            .transpose(1, 2, 0, 3)
            .astype(np_mm)
        )

    xq, xkv, mb = [], [], []
    for b in range(B):
        xq.append(to_dev(qs[b].T, QCH))
        if compact:
            idx = np.nonzero(mask[b] != 0)[0]
            sel = np.concatenate(
                [idx, np.zeros(C - len(idx), dtype=idx.dtype)]
            )
            bias = np.full(C, NEG, dtype=np.float32)
            bias[: len(idx)] = 0.0
            xkv.append(to_dev(qs[b][sel].T, 128))
        else:
            bias = np.where(mask[b] != 0, 0.0, NEG).astype(np.float32)
            xkv.append(to_dev(qs[b].T, 128))
        mb.append(np.ascontiguousarray(bias.reshape(C // 128, 128).T))

    in_maps = []
    for b in range(B):
        for g in range(HG):
            h0 = g * HL
            wq = Wqkv[:, (0 * H + h0) * A : (0 * H + h0 + HL) * A] * (
                1.0 / np.sqrt(np.float32(A))
            )
            wk = Wqkv[:, (1 * H + h0) * A : (1 * H + h0 + HL) * A]
            wv = Wqkv[:, (2 * H + h0) * A : (2 * H + h0 + HL) * A]
            # wqkv[p, blk, o, j] = blk_weights[o*128+p, j]
            wqkv_s = np.ascontiguousarray(
                np.stack([wq, wk, wv], axis=1)       # [D, 3, 256]
                .reshape(DSUB, 128, 3, HL * A)
                .transpose(1, 2, 0, 3)
                .astype(np_mm)
            )
            # wout[p, pair, d] = Wout_slice[pair*128 + p, d]
            wout_s = np.ascontiguousarray(
                Wout[h0 * A : (h0 + HL) * A, :]
                .reshape(NP, 128, D)
                .transpose(1, 0, 2)
                .astype(np_mm)
            )
            in_maps.append(
                {
                    "xq": xq[b],
                    "xkv": xkv[b],
                    "wqkv": wqkv_s,
                    "wout": wout_s,
                    "mbias": mb[b],
                }
            )
    return in_maps, C


def gather_output(results, bout):
    """Sum the 4 head-group partials per batch and add bout."""
    out = np.empty((B, S, D), dtype=np.float32)
    for b in range(B):
        acc = results[b * HG]["out"].astype(np.float32).copy()
        for g in range(1, HG):
            acc += results[b * HG + g]["out"]
        out[b] = acc + bout.astype(np.float32)[None, :]
    return out


def _ensure_ntff_hook():
    """Inject antenv.axon_hooks (missing on this image) so trace=True works."""
    import sys
    import types

    try:
        from antenv import axon_hooks  # noqa: F401
        return
    except ImportError:
        pass
    mod = types.ModuleType("antenv.axon_hooks")
    _h = [None]
    mod.set_axon_ntff_profile_hook = lambda h: _h.__setitem__(0, h)
    mod.get_axon_ntff_profile_hook = lambda: _h[0]
    sys.modules["antenv.axon_hooks"] = mod
    import antenv

    antenv.axon_hooks = mod
    try:
        from trn_agent_boot.trn_boot import _ntff_profile_via_ctypes

        mod.set_axon_ntff_profile_hook(
            _ntff_profile_via_ctypes("/opt/axon/libaxon_pjrt.so")
        )
    except Exception:
        pass


def run(qs, mask, Wqkv, Wout, bout, trace=False):
    if trace:
        _ensure_ntff_hook()
    in_maps, C = prepare_in_maps(qs, mask, Wqkv, Wout)
    nc = _get_program(C)
    res = run_bass_kernel_spmd(
        nc, in_maps, core_ids=list(range(B * HG)), trace=trace
    )
    return gather_output(res.results, np.asarray(bout)), res


def kernel(qs, mask, Wqkv, Wout, bout):
    return run(qs, mask, Wqkv, Wout, bout, trace=False)[0]
